# revision 35
# baseline (speedup 1.0000x reference)
"""Bass/Trainium2 kernel for nn_BaseAttention (B=2, N=2048, D=1024, H=16 causal).

Sharding: tensor-parallel over heads (2 heads/core on 8 cores).
Each core computes q/k/v projections for its 128-feature slice from the full
(pre-transposed) x, runs causal attention for its 2 heads, applies its slice
of the output projection (row-parallel Wo), and writes a full-shape partial
output (bf16). Partials are summed on the host in f32.

Device dataflow (per core, all matmuls bf16 -> fp32 PSUM):
  xT[kc] (SBUF)  --matmul-->  qT, kT   [128 feats, 4096 toks]   (feat-major)
                 --matmul-->  vT -> PE-transpose -> V [tok, feat]
  S^T[k,q] = kT_h.T @ qT_h   directly in transposed layout (no P transpose)
  E^T = exp(S^T)             (no max subtraction: |logits| < ~4 for this data)
  [O_h; den_h] = [V_h | 1].T @ E_h^T   per head, base partition 0
  1/den = exp(-ln(den)) on ACT over the den row, broadcast to partitions
  0..63 with a PE outer-product (no DRAM roundtrip); head 1's normalized
  rows shift to partitions 64..127 via a small sync-queue HWDGE DMA (the
  gpsimd/SWDGE path measured ~10x slower per DMA on this hardware).

Schedule: projection chunk nt is interleaved with attention chunk
(b, qc = nt - 4b) -- causality means chunk qc only needs k/v tiles up to
token (qc+1)*512 of its batch -- so projection matmuls fill the PE bubbles
of the ACT-paced attention pipeline and vice versa (sim: 200us -> 171us).

Measured (8-core SPMD, interleaved loop-slope): ~186 us/iteration; the
baseline this session started from measured ~192 us with the same
methodology (earlier 655 us figures were axon-dispatch-drift artifacts).
"""

import contextlib
import numpy as np
import ml_dtypes

B, N, D = 2, 2048, 1024
H, DH = 16, 64
NCORES = 8
HPC = H // NCORES        # heads per core
F = HPC * DH             # feature slice per core
F3 = 3 * F
T = B * N                # total tokens
SCALE = DH ** -0.5
P = 128
KC = D // P              # k-chunks over the model dim
NT = T // 512            # 512-token chunks
QC = N // 512            # q chunks per batch
JT = N // P              # 128-token k tiles per batch
VW = 2 * DH + 2          # va tile width: V_h0 | ones | V_h1 | ones

BF16 = ml_dtypes.bfloat16

_BUILT = {}


def _build_program(loop=0, opts=None):
    opts = dict(opts or {})
    import concourse.tile as tile
    from concourse import mybir
    from concourse.bacc import Bacc
    from concourse.masks import make_identity

    f32 = mybir.dt.float32
    bf16 = mybir.dt.bfloat16
    EXP = mybir.ActivationFunctionType.Exp
    LN = mybir.ActivationFunctionType.Ln

    class BaccOneActTable(Bacc):
        """Force every activation onto the natural_log_exp_and_others table
        set (it contains Exp, Ln, Copy and Identity) so the ACT engine loads
        its function table exactly once instead of thrashing between the
        exp and ln sets (~2.7us per reload)."""

        def insert_act_table_loads(self):
            import bass_rust as _bass_rust
            from concourse.hw_specs import get_activation_tables

            has_activation = any(
                isinstance(i, mybir.InstActivation)
                for blk in self.main_func.blocks
                for i in blk.instructions
            )
            if not has_activation:
                return
            keep = "natural_log_exp_and_others"
            tables = [
                (nm, (fns if nm == keep else set()))
                for nm, fns in get_activation_tables(self.m.arch).items()
            ]
            _bass_rust.insert_act_table_loads(self, tables)

    nc = BaccOneActTable()
    xt = nc.declare_dram_parameter("xt", [D, T], bf16, isOutput=False)
    w3 = nc.declare_dram_parameter("w3", [D, F3], bf16, isOutput=False)
    wo = nc.declare_dram_parameter("wo", [F, D], bf16, isOutput=False)
    out = nc.declare_dram_parameter("out", [T, D], bf16, isOutput=True)

    with tile.TileContext(nc) as tc:
        with contextlib.ExitStack() as ctx:
            persist = ctx.enter_context(tc.tile_pool(name="persist", bufs=1))
            work = ctx.enter_context(tc.tile_pool(name="work", bufs=1))

            # ---- persistent SBUF tensors ----
            xt_sb = persist.tile([P, KC, T], bf16)          # x^T, chunked over D
            w3_sb = persist.tile([P, KC, F3], bf16)         # wq|wk|wv packed
            wo_sb = persist.tile([P, D], bf16)
            wo2_sb = persist.tile([DH, D], bf16)            # wo rows 64:128 @ base 0
            qt_sb = persist.tile([P, T], bf16)              # Q^T (scaled)
            kt_sb = persist.tile([P, T], bf16)              # K^T
            # V per 128-tok tile: [0:64]=V_h0, 64=ones, [65:129]=V_h1, 129=ones
            va_sb = persist.tile([P, T // P, VW], bf16)
            ot_sb = persist.tile([P, T], bf16)              # normalized O^T
            et_all = persist.tile([P, 4, 1024], bf16)       # E^T rotation bufs
            dent2 = persist.tile([P, 1024], f32)            # ln(den) row 64
            rb16 = persist.tile([P, 1024], bf16)            # 1/den in bf16
            ident = persist.tile([P, P], bf16)
            # causal keep-mask for the diagonal 128x128 block: 1 iff c >= p
            trimask = persist.tile([P, P], bf16)
            # ones row at partition 64 for the den-recip broadcast outer
            onesbc = persist.tile([P, P], bf16)

            # ---- constants ----
            make_identity(nc, ident)
            nc.gpsimd.memset(trimask, 1.0)
            nc.gpsimd.affine_select(
                out=trimask, in_=trimask,
                compare_op=mybir.AluOpType.is_ge,
                fill=0.0, base=0, pattern=[[1, P]], channel_multiplier=-1,
            )
            nc.gpsimd.memset(onesbc[DH:DH + 1, 0:DH], 1.0)
            nc.gpsimd.memset(va_sb[:, :, DH], 1.0)
            nc.gpsimd.memset(va_sb[:, :, 2 * DH + 1], 1.0)
            nc.gpsimd.memset(et_all, 0.0)

            psum = tc.alloc_tile_pool(name="psum", bufs=1, space="PSUM")

            def body():
                # ---- load inputs: 6 DMAs spread over the queues ----
                w3_r = w3.rearrange("(a p) f -> p a f", p=P)
                if opts.get("w3_4way", False):
                    for i, q in enumerate((nc.scalar, nc.sync,
                                           nc.scalar, nc.sync)):
                        q.dma_start(out=w3_sb[:, 2 * i:2 * i + 2, :],
                                    in_=w3_r[:, 2 * i:2 * i + 2, :])
                else:
                    nc.scalar.dma_start(out=w3_sb[:, 0:KC // 2, :],
                                        in_=w3_r[:, 0:KC // 2, :])
                    nc.sync.dma_start(out=w3_sb[:, KC // 2:KC, :],
                                      in_=w3_r[:, KC // 2:KC, :])
                # xt bulk loads on the ACT HWDGE queue: measured ~7% faster
                # than the gpsimd/SWDGE queue; out stores stay on gpsimd+sync
                # (scalar-queue stores collide with the exp pipeline)
                xtq = nc.gpsimd if opts.get("xtgp") else nc.scalar
                xt_r = xt.rearrange("(a p) t -> p a t", p=P)
                nc.sync.dma_start(out=xt_sb[:, :, 0:512],
                                  in_=xt_r[:, :, 0:512])
                xtq.dma_start(out=xt_sb[:, :, 512:1024],
                              in_=xt_r[:, :, 512:1024])
                for g, q in ((1, xtq), (2, nc.sync), (3, xtq)):
                    q.dma_start(
                        out=xt_sb[:, :, g * 1024:(g + 1) * 1024],
                        in_=xt_r[:, :, g * 1024:(g + 1) * 1024])
                nc.scalar.dma_start(out=wo_sb, in_=wo[:, :])
                if opts.get("fuse_last", True):
                    nc.scalar.dma_start(out=wo2_sb, in_=wo[DH:P, :])

                # ---- phase emitters ----
                def emit_proj(nt):
                    c0 = nt * 512
                    pq = psum.tile([P, 1024], f32, tag="st", bufs=2,
                                   name=f"pq{nt}")
                    for kc in range(KC):
                        nc.tensor.matmul(
                            pq[:, 0:512], w3_sb[:, kc, 0:F],
                            xt_sb[:, kc, c0:c0 + 512],
                            start=(kc == 0), stop=(kc == KC - 1))
                    nc.scalar.copy(qt_sb[:, c0:c0 + 512], pq[:, 0:512])
                    pk = psum.tile([P, 1024], f32, tag="st", bufs=2,
                                   name=f"pk{nt}")
                    for kc in range(KC):
                        nc.tensor.matmul(
                            pk[:, 0:512], w3_sb[:, kc, F:2 * F],
                            xt_sb[:, kc, c0:c0 + 512],
                            start=(kc == 0), stop=(kc == KC - 1))
                    nc.vector.tensor_copy(kt_sb[:, c0:c0 + 512], pk[:, 0:512])
                    pvtag = "st" if opts.get("pv_st", False) else "ov0"
                    pv = psum.tile([P, 512], f32, tag=pvtag,
                                   bufs=2 if pvtag == "st" else 1,
                                   name=f"pv{nt}")
                    for kc in range(KC):
                        nc.tensor.matmul(
                            pv, w3_sb[:, kc, 2 * F:F3], xt_sb[:, kc, c0:c0 + 512],
                            start=(kc == 0), stop=(kc == KC - 1))
                    vt_tmp = work.tile([P, 512], bf16, tag="vt", bufs=2,
                                       name=f"vt{nt}")
                    nc.vector.tensor_copy(vt_tmp, pv)
                    for j4 in range(4):
                        tt = nt * 4 + j4
                        ptr = psum.tile([P, P], bf16, tag="wop", bufs=2,
                                        name=f"ptr{tt}")
                        nc.tensor.transpose(
                            ptr, vt_tmp[:, j4 * P:(j4 + 1) * P], ident)
                        nc.vector.tensor_copy(va_sb[:, tt, 0:DH], ptr[:, 0:DH])
                        nc.vector.tensor_copy(va_sb[:, tt, DH + 1:2 * DH + 1],
                                              ptr[:, DH:2 * DH])

                out_r = out.rearrange("(g f p) d -> p g f d", p=P, g=NT)
                if opts.get("nogp") or opts.get("outsc"):
                    outqs = (nc.scalar, nc.sync, nc.scalar, nc.sync)
                elif opts.get("outgp4"):
                    outqs = (nc.gpsimd, nc.gpsimd, nc.gpsimd, nc.gpsimd)
                else:
                    outqs = (nc.gpsimd, nc.sync, nc.gpsimd, nc.sync)

                def emit_wo(qg0, final=False):
                    wos4 = work.tile([P, 4, 1024], bf16, tag="wos4", bufs=2,
                                     name=f"wos4_{qg0}")
                    g = qg0 // 512
                    lotb = state.get("last_otb") if final else None
                    for ti in range(4):
                        t0 = qg0 + ti * P
                        for nn in range(2):
                            wop = psum.tile([P, 512], f32, tag="wop", bufs=2,
                                            name=f"wop{t0}_{nn}")
                            if lotb is not None:
                                nc.tensor.matmul(
                                    wop, ot_sb[0:DH, t0:t0 + P],
                                    wo_sb[0:DH, nn * 512:(nn + 1) * 512],
                                    start=True, stop=False)
                                nc.tensor.matmul(
                                    wop, lotb[:, ti * P:(ti + 1) * P],
                                    wo2_sb[:, nn * 512:(nn + 1) * 512],
                                    start=False, stop=True)
                            else:
                                nc.tensor.matmul(
                                    wop, ot_sb[:, t0:t0 + P],
                                    wo_sb[:, nn * 512:(nn + 1) * 512],
                                    start=True, stop=True)
                            weng = (nc.gpsimd.tensor_copy
                                    if opts.get("wos_pool", False) and nn == 1
                                    else nc.vector.tensor_copy)
                            weng(wos4[:, ti, nn * 512:(nn + 1) * 512], wop)
                        if final:
                            # per-tile store so the drain tail overlaps copies
                            outqs[ti].dma_start(
                                out=out_r[:, g, ti], in_=wos4[:, ti, :])
                    if not final:
                        # one batched store per 512 tokens, round-robin queues
                        outqs[g % 4].dma_start(out=out_r[:, g], in_=wos4)

                state = {"et_idx": 0, "pend": []}

                def emit_attn(b, qc):
                    qg = b * N + qc * 512     # global q offset
                    jmax = (qc + 1) * 4       # k tiles needed (causal)
                    # per-head [O; den] chains, both at base partition 0
                    ovs = [psum.tile([P, 512], f32, tag=f"ov{h}", bufs=1,
                                     name=f"ov{h}_{qg}")
                           for h in range(HPC)]
                    for h in range(HPC):
                        hp = DH * h           # feature offset of head
                        ov_mm = ovs[h][0:DH + 1, :]
                        va_c0 = h * (DH + 1)
                        va_c1 = va_c0 + DH + 1
                        for jj in range(0, jmax, 2):
                            st = psum.tile([P, 1024], f32, tag="st",
                                           bufs=2, name=f"st{qg}_{h}_{jj}")
                            et = et_all[:, state["et_idx"] % 4, :]
                            state["et_idx"] += 1
                            # pfx: fully-masked column prefix of the tile
                            pfx = [max(0, (jj + dj - qc * 4)) * P
                                   for dj in range(2)]
                            for dj in range(2):
                                j = jj + dj
                                kg = b * N + j * P
                                nc.tensor.matmul(
                                    st[:, dj * 512 + pfx[dj]:(dj + 1) * 512],
                                    kt_sb[hp:hp + DH, kg:kg + P],
                                    qt_sb[hp:hp + DH, qg + pfx[dj]:qg + 512],
                                    start=True, stop=True)
                            # exp the full tile even when a masked prefix
                            # exists: the prefix region of et is never read
                            # (EV and trimask slice past it)
                            if opts.get("exp_split", False):
                                for dj in range(2):
                                    nc.scalar.activation(
                                        et[:, dj * 512:(dj + 1) * 512],
                                        st[:, dj * 512:(dj + 1) * 512], EXP)
                            else:
                                nc.scalar.activation(et, st, EXP)
                            for dj in range(2):
                                j = jj + dj
                                if j >= qc * 4:   # diag: causal mask
                                    blk = slice(dj * 512 + pfx[dj],
                                                dj * 512 + pfx[dj] + P)
                                    nc.vector.tensor_mul(
                                        et[:, blk], et[:, blk], trimask)
                            for dj in range(2):
                                j = jj + dj
                                tt = b * JT + j
                                first = (jj == 0 and dj == 0)
                                last = (j == jmax - 1)
                                ets = et[:, dj * 512 + pfx[dj]:(dj + 1) * 512]
                                nc.tensor.matmul(
                                    ov_mm[:, pfx[dj]:512],
                                    va_sb[:, tt, va_c0:va_c1], ets,
                                    start=first, stop=last)
                    # move [O; den] out of PSUM; dens land on row 64 of
                    # each 512-col half of ovc
                    ovc = work.tile([P, 1024], f32, tag="ovc", bufs=2,
                                    name=f"ovc{qg}")
                    if opts.get("ovc_split", False):
                        nc.vector.tensor_copy(ovc[0:DH + 1, 0:512],
                                              ovs[0][0:DH + 1, :])
                        nc.scalar.copy(ovc[0:DH + 1, 512:1024],
                                       ovs[1][0:DH + 1, :])
                    else:
                        for h in range(HPC):
                            nc.vector.tensor_copy(
                                ovc[0:DH + 1, h * 512:(h + 1) * 512],
                                ovs[h][0:DH + 1, :])
                    # 1/den = exp(-ln(den)) on ACT (the custom-DVE
                    # reciprocal ops are broken on HW)
                    nc.scalar.activation(dent2[DH:DH + 1, :],
                                         ovc[DH:DH + 1, :], LN)
                    nc.scalar.activation(rb16[DH:DH + 1, :],
                                         dent2[DH:DH + 1, :],
                                         EXP, scale=-1.0)
                    # broadcast to partitions 0..63 via PE outer product
                    rbcs = []
                    for h in range(HPC):
                        rbc = psum.tile([P, 512], f32, tag="wop", bufs=2,
                                        name=f"rbc{qg}_{h}")
                        nc.tensor.matmul(
                            rbc[0:DH, :], onesbc[DH:DH + 1, 0:DH],
                            rb16[DH:DH + 1, h * 512:(h + 1) * 512],
                            start=True, stop=True)
                        rbcs.append(rbc)
                    nc.vector.tensor_mul(ot_sb[0:DH, qg:qg + 512],
                                         ovc[0:DH, 0:512], rbcs[0][0:DH, :])
                    otb = work.tile([DH, 512], bf16, tag="otb", bufs=2,
                                    name=f"otb{qg}")
                    nc.vector.tensor_mul(otb, ovc[0:DH, 512:1024],
                                         rbcs[1][0:DH, :])
                    # head 1 rows shift to partitions 64..127 via a
                    # scalar-queue HWDGE DMA (gpsimd/SWDGE is ~10x slower;
                    # the sync queue stalls these behind 1MB out-stores,
                    # costing ~36us/iter);
                    # the very last chunk skips the shift: its wo fuses two
                    # 64-contraction matmuls instead (tail latency)
                    last = (b == B - 1 and qc == QC - 1)
                    if opts.get("fuse_last", True) and last:
                        state["last_otb"] = otb
                    else:
                        otbq = nc.sync if opts.get("otbsync") else nc.scalar
                        otbq.dma_start(out=ot_sb[DH:P, qg:qg + 512], in_=otb)
                    # output projection, one qc late so the PE reaches
                    # these matmuls well after ot_sb is settled
                    state["pend"].append(qg)
                    delay = opts.get("wo_delay", 1)
                    while len(state["pend"]) > delay:
                        emit_wo(state["pend"].pop(0))

                # ---- interleaved schedule: proj chunk nt feeds attention
                # chunk (b, qc) with nt = b*QC + qc (causal: qc needs only
                # k/v tiles up to (qc+1)*512 tokens of its batch) ----
                if opts.get("interleave", True):
                    for b in range(B):
                        for qc in range(QC):
                            emit_proj(b * QC + qc)
                            emit_attn(b, qc)
                else:
                    for nt in range(NT):
                        emit_proj(nt)
                    for b in range(B):
                        for qc in range(QC):
                            emit_attn(b, qc)
                while len(state["pend"]) > 1:
                    emit_wo(state["pend"].pop(0))
                if state["pend"]:
                    emit_wo(state["pend"].pop(0), final=True)

            if loop:
                with tc.For_i(0, loop, 1):
                    body()
            else:
                for _ in range(opts.get("reps", 1)):
                    body()
            psum.release()

    nc.finalize()
    return nc


def _get_program():
    if "nc" not in _BUILT:
        _BUILT["nc"] = _build_program()
    return _BUILT["nc"]


def _prep_inputs(x, Wq, Wkv, Wo):
    xt = np.ascontiguousarray(x.reshape(T, D).T).astype(BF16)
    maps = []
    for c in range(NCORES):
        r0 = c * F
        w3 = np.concatenate([
            Wq[r0:r0 + F, :].T * SCALE,
            Wkv[r0:r0 + F, :].T,
            Wkv[D + r0:D + r0 + F, :].T,
        ], axis=1)
        maps.append({
            "xt": xt,
            "w3": np.ascontiguousarray(w3).astype(BF16),
            "wo": np.ascontiguousarray(Wo[:, r0:r0 + F].T).astype(BF16),
        })
    return maps


def kernel(x, Wq, Wkv, Wo):
    from concourse.bass_utils import run_bass_kernel_spmd

    nc = _get_program()
    in_maps = _prep_inputs(np.asarray(x, np.float32), np.asarray(Wq, np.float32),
                           np.asarray(Wkv, np.float32), np.asarray(Wo, np.float32))
    res = run_bass_kernel_spmd(nc, in_maps, list(range(NCORES)))
    acc = res.results[0]["out"].astype(np.float32)
    for c in range(1, NCORES):
        acc += res.results[c]["out"].astype(np.float32)
    return acc.reshape(B, N, D)
